# revision 1
# baseline (speedup 1.0000x reference)
"""Trainium2 Bass kernel for nn_CNN_56822417326399 (text-CNN forward).

Computation (per batch row b):
  E = emb[inp[b]]                      # [512, 300] gather
  conv = E @ conv_w.T + conv_b         # [512, 1000] (stride-D conv == per-token matmul)
  maxpool, idx = max/argmax over the 512 positions of relu(conv)  # per filter
  token[b, idx[f]] += maxpool[f] * (fc_w[1,f] - fc_w[0,f])        # scatter-add
  token += fc_b[1] - fc_b[0]

Sharding: data-parallel over batch, 16 rows per core on 8 cores; tables replicated.

Tricks:
- bias+relu commute with max over positions, so max/argmax run on the raw conv
  output and relu(max*2^-10 + bias) is applied to the per-filter scalar only
  (free on the ACT engine).  A clamped-to-0 max contributes 0, so its argmax
  position is harmless.
- conv runs as 3 fp16 matmul passes (hi*hi + lo*hi + hi*lo) on a 2^5-prescaled
  problem: emb and W are scaled by 32 on the host so the fp16 "lo" residual
  planes stay in normal fp16 range.  Residual error ~2^-23, fp32-equivalent,
  at 1 PE cycle/column instead of fp32 matmul's 4.
- argmax uses the DVE max/max_index instructions (exact first-occurrence
  semantics, matching jnp.argmax ties on duplicate tokens).
- the scatter-add is a fp16 matmul: token[1,512] += contrib[128f,1].T @
  onehot[128f,512], accumulated over the 8 filter tiles in PSUM.
"""

import numpy as np

B, L, D, V, F = 128, 512, 300, 50000, 1000
NCORES = 8
RPC = B // NCORES           # rows per core = 16
NRG = RPC // 4              # row groups of 4 rows = 4
FT = [128] * 7 + [104]      # f-tile sizes (1000 = 7*128 + 104)
DC = [128, 128, 44]         # d-chunk sizes (300 = 128 + 128 + 44)
SCALE = 32.0                # fp16 hi/lo prescale (2^5); conv is scaled by 2^10

_CACHE = {}


def _build_module(repeat=1):
    import concourse.tile as tile
    import concourse.mybir as mybir
    from concourse import bacc
    from concourse.bass import IndirectOffsetOnAxis, ts
    from concourse.masks import make_identity
    from contextlib import ExitStack

    f32 = mybir.dt.float32
    f16 = mybir.dt.float16
    i32 = mybir.dt.int32
    u32 = mybir.dt.uint32

    nc = bacc.Bacc("TRN2", target_bir_lowering=False, debug=False, num_devices=NCORES)

    emb_d = nc.dram_tensor("emb", [V, D], f32, kind="ExternalInput")  # pre-scaled x32
    wth_d = nc.dram_tensor("wth", [D, F], f16, kind="ExternalInput")  # fp16(W*32)
    wtl_d = nc.dram_tensor("wtl", [D, F], f16, kind="ExternalInput")  # residual
    wsk_d = nc.dram_tensor("wstk", [88, F], f16, kind="ExternalInput")  # [Wl_j2;Wh_j2]
    fc_d = nc.dram_tensor("fconst", [F, 2], f32, kind="ExternalInput")
    bd_d = nc.dram_tensor("biasd", [1, 1], f32, kind="ExternalInput")
    idx_d = nc.dram_tensor("idx", [128, RPC * 4], i32, kind="ExternalInput")
    out_d = nc.dram_tensor("out", [RPC, L], f32, kind="ExternalOutput")

    with tile.TileContext(nc) as tc, ExitStack() as ctx:
        const = ctx.enter_context(tc.tile_pool(name="const", bufs=1))
        e_pool = ctx.enter_context(tc.tile_pool(name="e", bufs=3))
        eT_pool = ctx.enter_context(tc.tile_pool(name="eT", bufs=3))
        oh_pool = ctx.enter_context(tc.tile_pool(name="oh", bufs=2))
        small = ctx.enter_context(tc.tile_pool(name="small", bufs=6))
        tokp = ctx.enter_context(tc.tile_pool(name="tok", bufs=4))
        psT = ctx.enter_context(tc.tile_pool(name="psT", bufs=3, space="PSUM"))
        psC = ctx.enter_context(tc.tile_pool(name="psC", bufs=4, space="PSUM"))
        psK = ctx.enter_context(tc.tile_pool(name="psK", bufs=1, space="PSUM"))

        ident = const.tile([128, 128], f16)
        make_identity(nc, ident[:])
        iota_i = const.tile([128, L], i32)
        nc.gpsimd.iota(iota_i[:], [[1, L]], channel_multiplier=0)
        iota_h = const.tile([128, L], f16)
        nc.vector.tensor_copy(iota_h[:], iota_i[:])

        idx_sb = const.tile([128, RPC * 4], i32)
        nc.sync.dma_start(idx_sb[:], idx_d[:])
        wt_sb = []  # [s][j] -> [128, F] fp16, s=0 hi, s=1 lo
        for s, wd in enumerate([wth_d, wtl_d]):
            tiles = []
            for j in range(3):
                w = const.tile([128, F], f16, tag=f"wt{s}{j}")
                dsz = DC[j]
                nc.sync.dma_start(w[0:dsz, :], wd[j * 128 : j * 128 + dsz, :])
                tiles.append(w)
            wt_sb.append(tiles)
        wsk_sb = const.tile([128, F], f16, tag="wsk")
        nc.sync.dma_start(wsk_sb[0:88, :], wsk_d[:])
        fc_sb = []
        for ft in range(8):
            fs = FT[ft]
            t = const.tile([128, 2], f32, tag=f"fc{ft}")
            nc.sync.dma_start(t[0:fs, :], fc_d[ft * 128 : ft * 128 + fs, :])
            fc_sb.append(t)
        bd_sb = const.tile([1, 1], f32)
        nc.sync.dma_start(bd_sb[:], bd_d[:])

        relu = mybir.ActivationFunctionType.Relu

        def body():
            for r in range(RPC):
                e_t = e_pool.tile([128, 4, D], f32, tag="e")
                for c in range(4):
                    nc.gpsimd.indirect_dma_start(
                        out=e_t[:, c, :],
                        out_offset=None,
                        in_=emb_d[:],
                        in_offset=IndirectOffsetOnAxis(
                            ap=idx_sb[:, r * 4 + c : r * 4 + c + 1], axis=0
                        ),
                    )
                # fp16 hi/lo split; free layout per token:
                # [0:300 hi | 300:344 lo_j2 | 344:472 lo_j0 | 472:600 lo_j1]
                # so that [Eh_j2 | El_j2] is contiguous at 256:344
                e_hl = e_pool.tile([128, 4, 2 * D], f16, tag="ehl")
                nc.scalar.copy(e_hl[:, :, 0:D], e_t[:])
                nc.gpsimd.tensor_tensor(
                    out=e_hl[:, :, 344:600],
                    in0=e_t[:, :, 0:256],
                    in1=e_hl[:, :, 0:256],
                    op=mybir.AluOpType.subtract,
                )
                nc.gpsimd.tensor_tensor(
                    out=e_hl[:, :, 300:344],
                    in0=e_t[:, :, 256:300],
                    in1=e_hl[:, :, 256:300],
                    op=mybir.AluOpType.subtract,
                )
                if True:
                    # eT[d, j, s, t]: transposed embeddings, s=0 hi / s=1 lo
                    eT = eT_pool.tile([128, 3, 2, L], f16, tag="eT")
                    for j in range(2):
                        pT = psT.tile([128, 2, L], f16, tag="pT")
                        for k in range(4):
                            nc.tensor.transpose(
                                out=pT[0:128, 0, ts(k, 128)],
                                in_=e_hl[:, k, j * 128 : (j + 1) * 128],
                                identity=ident[:],
                            )
                            nc.tensor.transpose(
                                out=pT[0:128, 1, ts(k, 128)],
                                in_=e_hl[:, k, 344 + j * 128 : 344 + (j + 1) * 128],
                                identity=ident[:],
                            )
                        nc.scalar.copy(eT[0:128, j, :, :], pT[0:128, :, :])
                    # j2: one [128, 88] transpose per k gives [Eh_j2; El_j2]
                    pT = psT.tile([128, 2, L], f16, tag="pT")
                    for k in range(4):
                        nc.tensor.transpose(
                            out=pT[0:88, 0, ts(k, 128)],
                            in_=e_hl[:, k, 256:344],
                            identity=ident[:],
                        )
                    nc.scalar.copy(eT[0:88, 2, 0, :], pT[0:88, 0, :])

                    tok_ps = psK.tile([1, L], f32, tag="tk")
                    cts, ohs = [], []
                    for ft in range(8):
                        fs = FT[ft]
                        cps = psC.tile([128, L], f32, tag="c")
                        fsl = slice(ft * 128, ft * 128 + fs)
                        passes = []
                        for j in range(2):
                            for ws, es in ((0, 0), (1, 0), (0, 1)):
                                passes.append(
                                    (wt_sb[ws][j][0:128, fsl], eT[0:128, j, es, :])
                                )
                        passes.append((wt_sb[0][2][0:44, fsl], eT[0:44, 2, 0, :]))
                        passes.append((wsk_sb[0:88, fsl], eT[0:88, 2, 0, :]))
                        for mm, (lw, re) in enumerate(passes):
                            nc.tensor.matmul(
                                out=cps[0:fs, :],
                                lhsT=lw,
                                rhs=re,
                                start=(mm == 0),
                                stop=(mm == len(passes) - 1),
                            )
                        max8 = small.tile([128, 8], f32, tag="m8")
                        nc.vector.max(out=max8[0:fs, :], in_=cps[0:fs, :])
                        idx8 = small.tile([128, 8], u32, tag="i8")
                        nc.vector.max_index(
                            out=idx8[0:fs, :],
                            in_max=max8[0:fs, :],
                            in_values=cps[0:fs, :],
                        )
                        idxh = small.tile([128, 1], f32, tag="ih")
                        nc.scalar.copy(idxh[0:fs, :], idx8[0:fs, 0:1])
                        # maxpool = relu(max * 2^-10 + conv_b)   (descale folded in)
                        mp = small.tile([128, 1], f32, tag="mp")
                        nc.scalar.activation(
                            mp[0:fs, :],
                            max8[0:fs, 0:1],
                            relu,
                            bias=fc_sb[ft][0:fs, 0:1],
                            scale=1.0 / (SCALE * SCALE),
                        )
                        ct = small.tile([128, 1], f16, tag=f"ct{ft}")
                        nc.scalar.mul(ct[0:fs, :], mp[0:fs, :], fc_sb[ft][0:fs, 1:2])
                        oh = oh_pool.tile([128, L], f16, tag=f"oh{ft}")
                        nc.vector.tensor_scalar(
                            out=oh[0:fs, :],
                            in0=iota_h[0:fs, :],
                            scalar1=idxh[0:fs, 0:1],
                            scalar2=None,
                            op0=mybir.AluOpType.is_equal,
                        )
                        cts.append(ct)
                        ohs.append(oh)
                    for ft in range(8):
                        fs = FT[ft]
                        nc.tensor.matmul(
                            out=tok_ps[0:1, :],
                            lhsT=cts[ft][0:fs, 0:1],
                            rhs=ohs[ft][0:fs, :],
                            start=(ft == 0),
                            stop=(ft == 7),
                        )
                    tok_sb = tokp.tile([1, L], f32, tag="ts")
                    nc.vector.tensor_scalar_add(
                        tok_sb[0:1, :], tok_ps[0:1, :], bd_sb[0:1, 0:1]
                    )
                    nc.sync.dma_start(out_d[r : r + 1, :], tok_sb[0:1, :])

        if repeat == 1:
            body()
        else:
            with tc.For_i(0, repeat, 1):
                body()

    nc.compile()
    return nc


def _get_module(repeat=1):
    key = ("mod", repeat)
    if key not in _CACHE:
        _CACHE[key] = _build_module(repeat)
    return _CACHE[key]


def _prep_inputs(inp, emb, conv_w, conv_b, fc_w, fc_b):
    inp = np.asarray(inp).astype(np.int32)
    emb = np.ascontiguousarray(np.asarray(emb, dtype=np.float32) * np.float32(SCALE))
    wt = np.ascontiguousarray(
        np.asarray(conv_w, dtype=np.float32)[:, 0, :].T * np.float32(SCALE)
    )
    wth = wt.astype(np.float16)
    wtl = (wt - wth.astype(np.float32)).astype(np.float16)
    # stacked cross weights for the K=88 j2 pass: [Wl_j2; Wh_j2]
    # (pairs with rhs [Eh_j2; El_j2] -> Wl*Eh + Wh*El)
    wstk = np.concatenate([wtl[256:300], wth[256:300]], axis=0)
    fc_w = np.asarray(fc_w, dtype=np.float32)
    fconst = np.ascontiguousarray(
        np.stack(
            [np.asarray(conv_b, dtype=np.float32), fc_w[1] - fc_w[0]], axis=1
        )
    )
    bd = np.array([[np.float32(fc_b[1]) - np.float32(fc_b[0])]], dtype=np.float32)
    in_maps = []
    for c in range(NCORES):
        rows = inp[c * RPC : (c + 1) * RPC]  # [16, 512]
        # idx[p, rg*16 + q*4 + k] = rows[rg*4 + q, 128*k + p]
        idx = np.ascontiguousarray(
            rows.reshape(NRG, 4, 4, 128).transpose(3, 0, 1, 2).reshape(128, RPC * 4)
        )
        in_maps.append(
            {
                "emb": emb,
                "wth": wth,
                "wtl": wtl,
                "wstk": wstk,
                "fconst": fconst,
                "biasd": bd,
                "idx": idx,
            }
        )
    return in_maps


def kernel(inp, emb, conv_w, conv_b, fc_w, fc_b):
    from concourse.bass_utils import run_bass_kernel_spmd

    in_maps = _prep_inputs(inp, emb, conv_w, conv_b, fc_w, fc_b)
    nc = _get_module()
    res = run_bass_kernel_spmd(nc, in_maps, core_ids=list(range(NCORES)))
    out = np.concatenate([res.results[c]["out"] for c in range(NCORES)], axis=0)
    return out.astype(np.float32)

